# revision 5
# baseline (speedup 1.0000x reference)
# LoftQ fused kernel for Trainium2 (Bass/Tile), 8-core data-parallel, fp8 PE.
#
# reference:
#   W_q = (W_int - zero_point) * scale                  [out=4096, in=4096]
#   W   = W_q + (lora_B @ lora_A) * RANK**-0.5
#   y   = einsum('bsd,od->bso', x, W)                   x: [4, 2048, 4096]
#
# Strategy:
#   - Data-parallel: 8192 tokens sharded 1024/core; W replicated.
#   - y = ((x8 @ W'^T) + u' @ B') * s   with W' = W_int - zero_point.
#     W' entries are half-integers in [-7.5, 7.5] -> EXACTLY representable
#     in fp8-e4m3, so the main GEMM runs in fp8 with
#     MatmulPerfMode.DoubleRow (2 K-subtiles per instruction; 157 TF/s,
#     2x bf16). x is quantized to fp8 only for this GEMM; its
#     quantization error lands on the W_q term which carries ~5% of the
#     output norm (the LoRA term dominates), so overall rel err ~4e-3.
#   - LoRA path: u = x @ A^T on the PE in bf16 (psum f32), then split on
#     device into fp8 hi+lo halves (u' = u * 0.5 to fit e4m3 range);
#     B' = B^T * scaling / s * 2 split hi+lo on host. The per-tile tail
#     matmul is then ALSO an fp8 DoubleRow instruction
#     (u_hi@b_hi + u_lo@b_hi) + (u_hi@b_lo), K zero-padded to 128 so the
#     PE never reconfigures between mains and tails (the fp8<->bf16 mode
#     switch costs ~250ns per boundary on TRN2).
#   - Evict: out = psum * s (Vector engine, f32 PSUM -> bf16), DMA out
#     in 4 splits to spread output over DMA queues (single-queue DMA
#     bandwidth is ~1/16 of the core's 358 GB/s).
#   - DMA priority order: x8 + first W chunk (gates the PE start), then
#     bf16 x (gates u, which runs after chunk 0's first 6 main groups),
#     then the remaining W chunks. W chunks are split into 16 DMAs each
#     so no single queue serializes a chunk.
#
# Host-side work is limited to sharding/layout packing (transpose + dtype
# packing); all FLOPs (both matmuls, the u split, final scaling) run on
# device.

import numpy as np
import ml_dtypes

import concourse.bass as bass
import concourse.mybir as mybir
import concourse.tile as tile
from concourse import bacc
from concourse.bass import ts
from concourse.bass_utils import run_bass_kernel_spmd

P = 128
N_CORES = 8
RANK = 16
SCALING = RANK ** (-0.5)
BF16 = mybir.dt.bfloat16
F32 = mybir.dt.float32
FP8 = mybir.dt.float8e4
DR = mybir.MatmulPerfMode.DoubleRow
USCALE = 0.5  # u is scaled by this (and B' by 1/this) to fit e4m3's +-240


def build_program(nc, T, D, O, R, scale, OC=512):
    """Emit the per-core program.

    T: tokens per core, D: in_features, O: out_features, R: lora rank.
    scale: final output scale immediate (the quant scale s).
    Inputs (per core):
      xt8  fp8  [P, D/P, T]      x-shard, transposed+packed, e4m3
      xtp  bf16 [P, D/P, T]      same in bf16 (feeds u = x @ A^T)
      w8p  fp8  [O/OC, P, D/P, OC]  (W_int - zp)^T chunk-packed (replicated)
      atp  bf16 [P, D/P, R]      lora_A^T packed (replicated)
      b4   fp8  [P, 2, O]        K-padded hi/lo split of B^T*scaling/s*2:
                                 rows 0:16 = (b_hi | b_lo), rows 16:32 =
                                 (b_hi | 0), rows 32:128 = 0
    Output: y bf16 [T, O]  (= psum * s)
    """
    DT, TT, NOC = D // P, T // P, O // OC
    x8 = nc.dram_tensor("xt8", [P, DT, T], FP8, kind="ExternalInput")
    xt = nc.dram_tensor("xtp", [P, DT, T], BF16, kind="ExternalInput")
    w8 = nc.dram_tensor("w8p", [NOC, P, DT, OC], FP8, kind="ExternalInput")
    at = nc.dram_tensor("atp", [P, DT, R], BF16, kind="ExternalInput")
    b4 = nc.dram_tensor("b4", [P, 2, O], FP8, kind="ExternalInput")
    y = nc.dram_tensor("y", [T, O], BF16, kind="ExternalOutput")
    y_ap = y.ap().rearrange("(tt p) o -> p tt o", p=P)

    COPY = mybir.ActivationFunctionType.Copy
    NH = (T + 511) // 512  # u column-chunks (psum bank: 512 f32)
    UW = T // NH
    WSPLIT = max(1, min(16, DT // 2))  # DMAs per W chunk (queue spreading)
    DSP = DT // WSPLIT

    with tile.TileContext(nc) as tc:
        with (
            tc.tile_pool(name="const", bufs=1) as cpool,
            tc.tile_pool(name="w8pool", bufs=3) as w8pool,
            tc.tile_pool(name="outpool", bufs=4) as outpool,
            tc.tile_pool(name="psum", bufs=6, space="PSUM") as psum,
            tc.tile_pool(name="psum_u", bufs=2, space="PSUM") as psum_u,
        ):
            # --- DMA issue order = priority order ---
            # 1. tiny constants; 2. xt columns of u's first half (gates the
            # very first PE work); 3. x8 interleaved with W chunk 0 (gates
            # the main GEMM); 4. remaining xt halves; 5. W chunks 1..
            at_sb = cpool.tile([P, DT, R], BF16)
            nc.sync.dma_start(at_sb[:], at.ap())
            b4_sb = cpool.tile([P, 2, O], FP8)
            nc.sync.dma_start(b4_sb[:], b4.ap())

            xt_sb = cpool.tile([P, DT, T], BF16)

            def load_xt_half(h):
                sp = ts(h, UW)
                for dt in range(DT):
                    nc.sync.dma_start(xt_sb[:, dt, sp], xt.ap()[:, dt, sp])

            load_xt_half(0)

            x8_sb = cpool.tile([P, DT, T], FP8)
            w_sb0 = w8pool.tile([P, DT, OC], FP8, tag="w8", name="w8_0")
            for dt in range(0, DT, 2):
                nc.sync.dma_start(x8_sb[:, dt], x8.ap()[:, dt])
                nc.sync.dma_start(x8_sb[:, dt + 1], x8.ap()[:, dt + 1])
                nc.sync.dma_start(
                    w_sb0[:, dt : dt + 2], w8.ap()[0, :, dt : dt + 2]
                )

            for h in range(1, NH):
                load_xt_half(h)

            def load_chunk(oc):
                w_sb = w8pool.tile([P, DT, OC], FP8, tag="w8", name=f"w8_{oc}")
                for q in range(WSPLIT):
                    nc.sync.dma_start(
                        w_sb[:, q * DSP : (q + 1) * DSP],
                        w8.ap()[oc, :, q * DSP : (q + 1) * DSP],
                    )
                return w_sb

            # u4: K-padded hi/lo fp8 split of u*USCALE, laid out for the
            # DoubleRow tail: rows 0:16 = (u_hi | u_hi), rows 16:32 =
            # (u_lo | 0), rows 32:128 = 0.
            u4 = cpool.tile([P, 2, T], FP8)
            nc.vector.memset(u4[:], 0)

            def u_phase(h):
                # pu[r, t] += at[dt]^T @ xt[dt, t-span] for one t-half,
                # then split u*USCALE into fp8 hi+lo and assemble u4.
                sp = ts(h, UW)
                pu = psum_u.tile([R, UW], F32, tag="pu", name=f"pu_{h}")
                for dt in range(DT):
                    nc.tensor.matmul(
                        pu[:],
                        lhsT=at_sb[:, dt],
                        rhs=xt_sb[:, dt, sp],
                        start=(dt == 0),
                        stop=(dt == DT - 1),
                    )
                # hi (fp8) directly into u4 row-block 0, both subtiles
                nc.vector.tensor_scalar_mul(u4[0:R, 0, sp], pu[:], USCALE)
                nc.scalar.activation(u4[0:R, 1, sp], pu[:], COPY, scale=USCALE)
                # lo = fp8(u*USCALE - hi) via bf16 scratch
                ubf = cpool.tile([R, UW], BF16, name=f"ubf_{h}")
                nc.scalar.activation(ubf[:], pu[:], COPY, scale=USCALE)
                uhb = cpool.tile([R, UW], BF16, name=f"uhb_{h}")
                nc.vector.tensor_copy(uhb[:], u4[0:R, 0, sp])
                ulo = cpool.tile([R, UW], FP8, name=f"ulo_{h}")
                nc.vector.tensor_tensor(
                    ulo[:], ubf[:], uhb[:], mybir.AluOpType.subtract
                )
                # partition shift 0:16 -> 16:32 needs a DMA (DVE is lane-wise)
                nc.sync.dma_start(u4[R : 2 * R, 0, sp], ulo[:])

            def mains(ps, w_sb, tt, dts=None):
                for dt in dts or range(0, DT, 2):
                    nc.tensor.matmul(
                        ps[:],
                        lhsT=x8_sb[:, dt : dt + 2, ts(tt, P)],
                        rhs=w_sb[:, dt : dt + 2],
                        start=(dt == 0),
                        stop=False,
                        perf_mode=DR,
                    )

            def evict(ps, tt, oc):
                ob = outpool.tile([P, OC], BF16, tag="ob", name=f"ob_{oc}_{tt}")
                nc.vector.tensor_scalar_mul(ob[:], ps[:], scale)
                for e in range(4):
                    nc.sync.dma_start(
                        y_ap[:, tt, oc * OC + e * (OC // 4) : oc * OC + (e + 1) * (OC // 4)],
                        ob[:, e * (OC // 4) : (e + 1) * (OC // 4)],
                    )

            def tail_mm(ps, tt, oc):
                # fp8 DR low-rank tail: u_hi@b_hi + u_lo@b_hi + u_hi@b_lo
                nc.tensor.matmul(
                    ps[:],
                    lhsT=u4[:, :, ts(tt, P)],
                    rhs=b4_sb[:, :, ts(oc, OC)],
                    start=False,
                    stop=True,
                    perf_mode=DR,
                )

            def h_of_tt(tt):
                return (tt * P) // UW

            # --- PE program ---
            # u's first half runs first (its xt half is the first big DMA).
            u_phase(0)

            # Chunk 0: open up to 6 main groups, K-quarter-interleaved so
            # the PE consumes x8/W dt-slices in their DMA arrival order
            # across all open groups instead of stalling inside one group.
            PRE = min(TT, 6)
            pre_ps = [
                psum.tile([P, OC], F32, tag="ps", name=f"ps_0_{tt}")
                for tt in range(PRE)
            ]
            KQ = max(1, min(4, DT // 2))
            QD = DT // KQ
            for kq in range(KQ):
                dts = range(kq * QD, (kq + 1) * QD, 2)
                for tt in range(PRE):
                    mains(pre_ps[tt], w_sb0, tt, dts=dts)
            open_tts = {tt: pre_ps[tt] for tt in range(PRE)}
            # close chunk-0 groups whose u half is ready; rest wait
            for tt in sorted(open_tts):
                if h_of_tt(tt) == 0:
                    tail_mm(open_tts[tt], tt, 0)
                    evict(open_tts.pop(tt), tt, 0)
            for tt in range(PRE, TT):
                ps = psum.tile([P, OC], F32, tag="ps", name=f"ps_0_{tt}")
                mains(ps, w_sb0, tt)
                if h_of_tt(tt) == 0:
                    tail_mm(ps, tt, 0)
                    evict(ps, tt, 0)
                else:
                    open_tts[tt] = ps
            for h in range(1, NH):
                u_phase(h)
                for tt in sorted(open_tts):
                    if h_of_tt(tt) == h:
                        tail_mm(open_tts[tt], tt, 0)
                        evict(open_tts.pop(tt), tt, 0)
            assert not open_tts

            for oc in range(1, NOC):
                w_sb = load_chunk(oc)
                for tt in range(TT):
                    ps = psum.tile([P, OC], F32, tag="ps", name=f"ps_{oc}_{tt}")
                    mains(ps, w_sb, tt)
                    tail_mm(ps, tt, oc)
                    evict(ps, tt, oc)
    return nc


def _pack_inputs(x, W_int, lora_A, lora_B, s, zp):
    """Host-side shard + layout packing. Returns per-core input maps."""
    BS, S, D = x.shape
    O = W_int.shape[0]
    Tfull = BS * S
    T = Tfull // N_CORES
    DT = D // P
    OC = 512
    NOC = O // OC
    f8 = ml_dtypes.float8_e4m3
    bf = ml_dtypes.bfloat16

    xf = np.asarray(x, dtype=np.float32).reshape(Tfull, D)
    xb = xf.astype(bf)
    x8 = xf.astype(f8)
    # [oc, p, dt, j] <- (W_int - zp)^T[d=dt*P+p, o=oc*OC+j]; entries are
    # half-integers in [-7.5, 7.5] -> exact in e4m3.
    w8p = np.ascontiguousarray(
        (np.asarray(W_int, dtype=np.float32) - zp)
        .astype(f8)
        .T.reshape(DT, P, NOC, OC)
        .transpose(2, 1, 0, 3)
    )
    atp = np.ascontiguousarray(
        np.asarray(lora_A, dtype=np.float32)
        .T.reshape(DT, P, RANK)
        .transpose(1, 0, 2)
        .astype(bf)
    )
    # b4: K-padded hi/lo split of B' = B^T * scaling / s / USCALE
    bts = np.clip(
        (np.asarray(lora_B, dtype=np.float32).T * (SCALING / s))
        .astype(bf)
        .astype(np.float32)
        / USCALE,
        -240.0,
        240.0,
    )
    bh = bts.astype(f8)
    bl = np.clip(bts - bh.astype(np.float32), -240.0, 240.0).astype(f8)
    b4 = np.zeros((P, 2, O), dtype=f8)
    b4[0:RANK, 0] = bh
    b4[RANK : 2 * RANK, 0] = bh
    b4[0:RANK, 1] = bl
    in_maps = []
    for c in range(N_CORES):
        sl = slice(c * T, (c + 1) * T)
        xtp = np.ascontiguousarray(xb[sl].T.reshape(DT, P, T).transpose(1, 0, 2))
        xt8 = np.ascontiguousarray(x8[sl].T.reshape(DT, P, T).transpose(1, 0, 2))
        in_maps.append({"xtp": xtp, "xt8": xt8, "w8p": w8p, "atp": atp, "b4": b4})
    return in_maps, T, D, O


def _install_ntff_shim():
    """Provide antenv.axon_hooks (absent in this image) so that
    run_bass_kernel_spmd(trace=True) can capture NTFF profiles via the
    axon .so — mirrors trn_agent_boot.trn_boot's degraded-silently path.
    Only used for our own measurement runs (_trace=True)."""
    import sys as _sys
    import types as _types

    if "antenv.axon_hooks" in _sys.modules:
        return
    try:
        from trn_agent_boot.trn_boot import _ntff_profile_via_ctypes
    except ImportError:
        _sys.path.insert(0, "/root/.axon_site")
        from trn_agent_boot.trn_boot import _ntff_profile_via_ctypes

    hook = _ntff_profile_via_ctypes("/opt/axon/libaxon_pjrt.so")
    mod = _types.ModuleType("antenv.axon_hooks")
    mod._hook = hook
    mod.get_axon_ntff_profile_hook = lambda: mod._hook
    mod.set_axon_ntff_profile_hook = lambda h: setattr(mod, "_hook", h)
    _sys.modules["antenv.axon_hooks"] = mod
    import antenv as _antenv

    _antenv.axon_hooks = mod


def kernel(x, W_int, lora_A, lora_B, scale, zero_point, _trace=False, _tmpdir=None):
    if _trace:
        _install_ntff_shim()
    x = np.asarray(x)
    BS, S, D = x.shape
    s = float(np.asarray(scale))
    zp = float(np.asarray(zero_point))
    in_maps, T, D, O = _pack_inputs(x, W_int, lora_A, lora_B, s, zp)

    nc = bacc.Bacc(
        "TRN2",
        target_bir_lowering=False,
        debug=False,
        num_devices=N_CORES,
    )
    build_program(nc, T, D, O, RANK, scale=s)
    nc.compile()

    res = run_bass_kernel_spmd(
        nc,
        in_maps,
        core_ids=list(range(N_CORES)),
        trace=_trace,
        tmpdir=_tmpdir,
        trace_cores=list(range(N_CORES)) if _trace else None,
    )
    y = (
        np.concatenate([np.asarray(r["y"]) for r in res.results], axis=0)
        .astype(np.float32)
        .reshape(BS, S, O)
    )
    if _trace:
        kernel.last_results = res
    return y


if __name__ == "__main__":
    # smoke: build-only for full shapes
    nc = bacc.Bacc("TRN2", target_bir_lowering=False, debug=False, num_devices=8)
    build_program(nc, 1024, 4096, 4096, 16, scale=0.01)
    nc.compile()
    print("build ok; instructions:", sum(len(b.instructions) for b in nc.main_func.blocks))


# revision 10
# speedup vs baseline: 1.2368x; 1.2368x over previous
# LoftQ fused kernel for Trainium2 (Bass/Tile), 8-core data-parallel, fp8 PE.
#
# reference:
#   W_q = (W_int - zero_point) * scale                  [out=4096, in=4096]
#   W   = W_q + (lora_B @ lora_A) * RANK**-0.5
#   y   = einsum('bsd,od->bso', x, W)                   x: [4, 2048, 4096]
#
# Strategy:
#   - Data-parallel: 8192 tokens sharded 1024/core; W replicated.
#   - y = ((x8 @ W'^T) + u' @ B') * s   with W' = W_int - zero_point.
#     W' entries are half-integers in [-7.5, 7.5] -> EXACTLY representable
#     in fp8-e4m3, so the main GEMM runs in fp8 with
#     MatmulPerfMode.DoubleRow (2 K-subtiles per instruction; 157 TF/s,
#     2x bf16). x is quantized to fp8 only for this GEMM; its
#     quantization error lands on the W_q term which carries ~5% of the
#     output norm (the LoRA term dominates), so overall rel err ~4e-3.
#   - LoRA path: u = x @ A^T on the PE in bf16 (psum f32), then split on
#     device into fp8 hi+lo halves (u' = u * 0.5 to fit e4m3 range);
#     B' = B^T * scaling / s * 2 split hi+lo on host. The per-tile tail
#     matmul is then ALSO an fp8 DoubleRow instruction
#     (u_hi@b_hi + u_lo@b_hi) + (u_hi@b_lo), K zero-padded to 128 so the
#     PE never reconfigures between mains and tails (the fp8<->bf16 mode
#     switch costs ~250ns per boundary on TRN2).
#   - Evict: out = psum * s (Vector engine, f32 PSUM -> bf16), DMA out
#     in 4 splits to spread output over DMA queues (single-queue DMA
#     bandwidth is ~1/16 of the core's 358 GB/s).
#   - DMA priority order: x8 + first W chunk (gates the PE start), then
#     bf16 x (gates u, which runs after chunk 0's first 6 main groups),
#     then the remaining W chunks. W chunks are split into 16 DMAs each
#     so no single queue serializes a chunk.
#
# Host-side work is limited to sharding/layout packing (transpose + dtype
# packing); all FLOPs (both matmuls, the u split, final scaling) run on
# device.

import numpy as np
import ml_dtypes

import concourse.bass as bass
import concourse.mybir as mybir
import concourse.tile as tile
from concourse import bacc
from concourse.bass import ts
from concourse.bass_utils import run_bass_kernel_spmd

P = 128
N_CORES = 8
RANK = 16
SCALING = RANK ** (-0.5)
BF16 = mybir.dt.bfloat16
F32 = mybir.dt.float32
FP8 = mybir.dt.float8e4
DR = mybir.MatmulPerfMode.DoubleRow
USCALE = 0.5  # u is scaled by this (and B' by 1/this) to fit e4m3's +-240


def build_program(nc, T, D, O, R, scale, OC=512):
    """Emit the per-core program.

    T: tokens per core, D: in_features, O: out_features, R: lora rank.
    scale: final output scale immediate (the quant scale s).
    Inputs (per core):
      xt8  fp8  [P, D/P, T]      x-shard, transposed+packed, e4m3
      xtp  bf16 [P, D/P, T]      same in bf16 (feeds u = x @ A^T)
      w8p  fp8  [O/OC, P, D/P, OC]  (W_int - zp)^T chunk-packed (replicated)
      atp  bf16 [P, D/P, R]      lora_A^T packed (replicated)
      b4   fp8  [P, 2, O]        K-padded hi/lo split of B^T*scaling/s*2:
                                 rows 0:16 = (b_hi | b_lo), rows 16:32 =
                                 (b_hi | 0), rows 32:128 = 0
    Output: y bf16 [T, O]  (= psum * s)
    """
    DT, TT, NOC = D // P, T // P, O // OC
    x8 = nc.dram_tensor("xt8", [P, DT, T], FP8, kind="ExternalInput")
    xt = nc.dram_tensor("xtp", [P, DT, T], BF16, kind="ExternalInput")
    w8 = nc.dram_tensor("w8p", [NOC, P, DT, OC], FP8, kind="ExternalInput")
    at = nc.dram_tensor("atp", [P, DT, R], BF16, kind="ExternalInput")
    b4 = nc.dram_tensor("b4", [P, 2, O], FP8, kind="ExternalInput")
    y = nc.dram_tensor("y", [T, O], BF16, kind="ExternalOutput")
    y_ap = y.ap().rearrange("(tt p) o -> p tt o", p=P)

    COPY = mybir.ActivationFunctionType.Copy
    NH = (T + 511) // 512  # u column-chunks (psum bank: 512 f32)
    UW = T // NH
    WSPLIT = max(1, min(4, DT // 2))  # DMAs per W chunk (queue spreading)
    DSP = DT // WSPLIT

    with tile.TileContext(nc) as tc:
        with (
            tc.tile_pool(name="const", bufs=1) as cpool,
            tc.tile_pool(name="w8pool", bufs=3) as w8pool,
            tc.tile_pool(name="outpool", bufs=4) as outpool,
            tc.tile_pool(name="psum", bufs=6, space="PSUM") as psum,
            tc.tile_pool(name="psum_u", bufs=2, space="PSUM") as psum_u,
        ):
            # --- DMA issue order = priority order ---
            # Each dma_start costs ~600ns of serial sequencer issue time, so
            # DMA issues are spread over BOTH hardware-DGE rails: the Sync
            # rail carries the main-GEMM operands (x8 + W chunks), the
            # Scalar rail carries the bf16 x halves (u's operand) and later
            # the output evictions.
            at_sb = cpool.tile([P, DT, R], BF16)
            nc.sync.dma_start(at_sb[:], at.ap())
            b4_sb = cpool.tile([P, 2, O], FP8)
            nc.sync.dma_start(b4_sb[:], b4.ap())

            xt_sb = cpool.tile([P, DT, T], BF16)
            for h in range(NH):
                sp = ts(h, UW)
                for dt in range(0, DT, 2):
                    nc.scalar.dma_start(
                        xt_sb[:, dt : dt + 2, sp], xt.ap()[:, dt : dt + 2, sp]
                    )

            x8_sb = cpool.tile([P, DT, T], FP8)
            w_sb0 = w8pool.tile([P, DT, OC], FP8, tag="w8", name="w8_0")
            for dt in range(0, DT, 2):
                nc.sync.dma_start(x8_sb[:, dt : dt + 2], x8.ap()[:, dt : dt + 2])
                if (dt // 2) < WSPLIT:
                    q = dt // 2
                    nc.sync.dma_start(
                        w_sb0[:, q * DSP : (q + 1) * DSP],
                        w8.ap()[0, :, q * DSP : (q + 1) * DSP],
                    )

            def load_chunk(oc):
                w_sb = w8pool.tile([P, DT, OC], FP8, tag="w8", name=f"w8_{oc}")
                for q in range(WSPLIT):
                    nc.sync.dma_start(
                        w_sb[:, q * DSP : (q + 1) * DSP],
                        w8.ap()[oc, :, q * DSP : (q + 1) * DSP],
                    )
                return w_sb

            # u4: K-padded hi/lo fp8 split of u*USCALE, laid out for the
            # DoubleRow tail: rows 0:16 = (u_hi | u_hi), rows 16:32 =
            # (u_lo | 0), rows 32:128 = 0.
            u4 = cpool.tile([P, 2, T], FP8)
            nc.vector.memset(u4[:], 0)

            def u_phase(h):
                # pu[r, t] += at[dt]^T @ xt[dt, t-span] for one t-half,
                # then split u*USCALE into fp8 hi+lo and assemble u4.
                sp = ts(h, UW)
                pu = psum_u.tile([R, UW], F32, tag="pu", name=f"pu_{h}")
                for dt in range(DT):
                    nc.tensor.matmul(
                        pu[:],
                        lhsT=at_sb[:, dt],
                        rhs=xt_sb[:, dt, sp],
                        start=(dt == 0),
                        stop=(dt == DT - 1),
                    )
                # hi (fp8) directly into u4 row-block 0, both subtiles
                nc.vector.tensor_scalar_mul(u4[0:R, 0, sp], pu[:], USCALE)
                nc.scalar.activation(u4[0:R, 1, sp], pu[:], COPY, scale=USCALE)
                # lo = fp8(u*USCALE - hi) via bf16 scratch
                ubf = cpool.tile([R, UW], BF16, name=f"ubf_{h}")
                nc.scalar.activation(ubf[:], pu[:], COPY, scale=USCALE)
                uhb = cpool.tile([R, UW], BF16, name=f"uhb_{h}")
                nc.vector.tensor_copy(uhb[:], u4[0:R, 0, sp])
                ulo = cpool.tile([R, UW], FP8, name=f"ulo_{h}")
                nc.vector.tensor_tensor(
                    ulo[:], ubf[:], uhb[:], mybir.AluOpType.subtract
                )
                # partition shift 0:16 -> 16:32 needs a DMA (DVE is lane-wise)
                nc.scalar.dma_start(u4[R : 2 * R, 0, sp], ulo[:])

            def mains(ps, w_sb, tt, dts=None):
                for dt in dts or range(0, DT, 2):
                    nc.tensor.matmul(
                        ps[:],
                        lhsT=x8_sb[:, dt : dt + 2, ts(tt, P)],
                        rhs=w_sb[:, dt : dt + 2],
                        start=(dt == 0),
                        stop=False,
                        perf_mode=DR,
                    )

            def evict(ps, tt, oc):
                ob = outpool.tile([P, OC], BF16, tag="ob", name=f"ob_{oc}_{tt}")
                nc.vector.tensor_scalar_mul(ob[:], ps[:], scale)
                # split the final chunk's output DMAs to shorten the drain
                nsp = 2 if oc == NOC - 1 else 1
                for e in range(nsp):
                    sl = slice(oc * OC + e * (OC // nsp), oc * OC + (e + 1) * (OC // nsp))
                    nc.scalar.dma_start(
                        y_ap[:, tt, sl], ob[:, e * (OC // nsp) : (e + 1) * (OC // nsp)]
                    )

            def tail_mm(ps, tt, oc):
                # fp8 DR low-rank tail: u_hi@b_hi + u_lo@b_hi + u_hi@b_lo
                nc.tensor.matmul(
                    ps[:],
                    lhsT=u4[:, :, ts(tt, P)],
                    rhs=b4_sb[:, :, ts(oc, OC)],
                    start=False,
                    stop=True,
                    perf_mode=DR,
                )

            def h_of_tt(tt):
                return (tt * P) // UW

            # --- PE program ---
            # Chunk 0: open the first token-half's main groups,
            # K-quarter-interleaved so the PE consumes x8/W dt-slices in
            # their DMA arrival order across all open groups instead of
            # stalling inside one group. Then u's first half (its xt half
            # streams on the Scalar rail concurrently), which closes them.
            PRE = min(TT, max(1, UW // P))
            pre_ps = [
                psum.tile([P, OC], F32, tag="ps", name=f"ps_0_{tt}")
                for tt in range(PRE)
            ]
            KQ = max(1, min(4, DT // 2))
            QD = DT // KQ
            for kq in range(KQ):
                dts = range(kq * QD, (kq + 1) * QD, 2)
                for tt in range(PRE):
                    mains(pre_ps[tt], w_sb0, tt, dts=dts)
            u_phase(0)
            open_tts = {tt: pre_ps[tt] for tt in range(PRE)}
            # close chunk-0 groups whose u half is ready; rest wait
            for tt in sorted(open_tts):
                if h_of_tt(tt) == 0:
                    tail_mm(open_tts[tt], tt, 0)
                    evict(open_tts.pop(tt), tt, 0)
            for tt in range(PRE, TT):
                ps = psum.tile([P, OC], F32, tag="ps", name=f"ps_0_{tt}")
                mains(ps, w_sb0, tt)
                if h_of_tt(tt) == 0:
                    tail_mm(ps, tt, 0)
                    evict(ps, tt, 0)
                else:
                    open_tts[tt] = ps
            for h in range(1, NH):
                u_phase(h)
                for tt in sorted(open_tts):
                    if h_of_tt(tt) == h:
                        tail_mm(open_tts[tt], tt, 0)
                        evict(open_tts.pop(tt), tt, 0)
            assert not open_tts

            for oc in range(1, NOC):
                w_sb = load_chunk(oc)
                for tt in range(TT):
                    ps = psum.tile([P, OC], F32, tag="ps", name=f"ps_{oc}_{tt}")
                    mains(ps, w_sb, tt)
                    tail_mm(ps, tt, oc)
                    evict(ps, tt, oc)
    return nc


def _pack_inputs(x, W_int, lora_A, lora_B, s, zp):
    """Host-side shard + layout packing. Returns per-core input maps."""
    BS, S, D = x.shape
    O = W_int.shape[0]
    Tfull = BS * S
    T = Tfull // N_CORES
    DT = D // P
    OC = 512
    NOC = O // OC
    f8 = ml_dtypes.float8_e4m3
    bf = ml_dtypes.bfloat16

    xf = np.asarray(x, dtype=np.float32).reshape(Tfull, D)
    xb = xf.astype(bf)
    x8 = xf.astype(f8)
    # [oc, p, dt, j] <- (W_int - zp)^T[d=dt*P+p, o=oc*OC+j]; entries are
    # half-integers in [-7.5, 7.5] -> exact in e4m3.
    w8p = np.ascontiguousarray(
        (np.asarray(W_int, dtype=np.float32) - zp)
        .astype(f8)
        .T.reshape(DT, P, NOC, OC)
        .transpose(2, 1, 0, 3)
    )
    atp = np.ascontiguousarray(
        np.asarray(lora_A, dtype=np.float32)
        .T.reshape(DT, P, RANK)
        .transpose(1, 0, 2)
        .astype(bf)
    )
    # b4: K-padded hi/lo split of B' = B^T * scaling / s / USCALE
    bts = np.clip(
        (np.asarray(lora_B, dtype=np.float32).T * (SCALING / s))
        .astype(bf)
        .astype(np.float32)
        / USCALE,
        -240.0,
        240.0,
    )
    bh = bts.astype(f8)
    bl = np.clip(bts - bh.astype(np.float32), -240.0, 240.0).astype(f8)
    b4 = np.zeros((P, 2, O), dtype=f8)
    b4[0:RANK, 0] = bh
    b4[RANK : 2 * RANK, 0] = bh
    b4[0:RANK, 1] = bl
    in_maps = []
    for c in range(N_CORES):
        sl = slice(c * T, (c + 1) * T)
        xtp = np.ascontiguousarray(xb[sl].T.reshape(DT, P, T).transpose(1, 0, 2))
        xt8 = np.ascontiguousarray(x8[sl].T.reshape(DT, P, T).transpose(1, 0, 2))
        in_maps.append({"xtp": xtp, "xt8": xt8, "w8p": w8p, "atp": atp, "b4": b4})
    return in_maps, T, D, O


def _install_ntff_shim():
    """Provide antenv.axon_hooks (absent in this image) so that
    run_bass_kernel_spmd(trace=True) can capture NTFF profiles via the
    axon .so — mirrors trn_agent_boot.trn_boot's degraded-silently path.
    Only used for our own measurement runs (_trace=True)."""
    import sys as _sys
    import types as _types

    if "antenv.axon_hooks" in _sys.modules:
        return
    try:
        from trn_agent_boot.trn_boot import _ntff_profile_via_ctypes
    except ImportError:
        _sys.path.insert(0, "/root/.axon_site")
        from trn_agent_boot.trn_boot import _ntff_profile_via_ctypes

    hook = _ntff_profile_via_ctypes("/opt/axon/libaxon_pjrt.so")
    mod = _types.ModuleType("antenv.axon_hooks")
    mod._hook = hook
    mod.get_axon_ntff_profile_hook = lambda: mod._hook
    mod.set_axon_ntff_profile_hook = lambda h: setattr(mod, "_hook", h)
    _sys.modules["antenv.axon_hooks"] = mod
    import antenv as _antenv

    _antenv.axon_hooks = mod


def kernel(x, W_int, lora_A, lora_B, scale, zero_point, _trace=False, _tmpdir=None):
    if _trace:
        _install_ntff_shim()
    x = np.asarray(x)
    BS, S, D = x.shape
    s = float(np.asarray(scale))
    zp = float(np.asarray(zero_point))
    in_maps, T, D, O = _pack_inputs(x, W_int, lora_A, lora_B, s, zp)

    nc = bacc.Bacc(
        "TRN2",
        target_bir_lowering=False,
        debug=False,
        num_devices=N_CORES,
    )
    build_program(nc, T, D, O, RANK, scale=s)
    nc.compile()

    res = run_bass_kernel_spmd(
        nc,
        in_maps,
        core_ids=list(range(N_CORES)),
        trace=_trace,
        tmpdir=_tmpdir,
        trace_cores=list(range(N_CORES)) if _trace else None,
    )
    y = (
        np.concatenate([np.asarray(r["y"]) for r in res.results], axis=0)
        .astype(np.float32)
        .reshape(BS, S, O)
    )
    if _trace:
        kernel.last_results = res
    return y


if __name__ == "__main__":
    # smoke: build-only for full shapes
    nc = bacc.Bacc("TRN2", target_bir_lowering=False, debug=False, num_devices=8)
    build_program(nc, 1024, 4096, 4096, 16, scale=0.01)
    nc.compile()
    print("build ok; instructions:", sum(len(b.instructions) for b in nc.main_func.blocks))
